# revision 26
# baseline (speedup 1.0000x reference)
"""NTM external-memory step (cosine addressing -> shift -> sharpen -> read/erase/add)
as a Bass/Tile SPMD kernel for 8 Trainium2 NeuronCores.

Sharding: mem (and all N-length address vectors) are sharded along the
mem_size axis across the 8 cores.  Each core keeps its 16 MiB mem shard
resident in SBUF between the addressing pass and the erase/add pass, so HBM
traffic is one read + one write of mem.  The softmax / sharpen
normalizations and the circular-shift halo are handled with two tiny
AllGather collectives (3 floats and 1 float per core).

Per-core layout (n_loc = mem_size/8 rows, P=128 partitions, T = n_loc/128):
  local row l = p*T + t  lives at SBUF partition p, free block t.
  Address vectors are [128, T] tensors; the +-1 circular shift is a free-dim
  offset except at per-partition block boundaries, which are patched via two
  PE transpose round-trips, and at core boundaries, which come from the
  AllGather halo.
"""

import sys
import functools

for _p in ("/opt/trn_rl_repo",):
    if _p not in sys.path:
        sys.path.insert(0, _p)

import numpy as np

from concourse import bass, bacc, mybir, tile
from concourse.bass_utils import run_bass_kernel_spmd

F32 = mybir.dt.float32
BF16 = mybir.dt.bfloat16
AF = mybir.ActivationFunctionType
ALU = mybir.AluOpType
X = mybir.AxisListType.X

N_CORES = 8
MEM_SIZE = 131072
MEM_WIDTH = 256
P = 128

# consts column map (consts input is [128, CONSTS_W], replicated across partitions)
C_ONE = 0
C_GAMMA = 1
C_S0 = 2
C_S1 = 3
C_S2 = 4
C_HGL = 5   # (1-gate)*clip01(w_prev[left halo row])
C_HGR = 6   # (1-gate)*clip01(w_prev[right halo row])
C_GATE = 7
C_SELL = 8                      # 3*N_CORES one-hot mask for left-neighbor E_last
C_SELR = 8 + 3 * N_CORES        # 3*N_CORES one-hot mask for right-neighbor E_first
C_ONESROW = 8 + 6 * N_CORES     # 128 columns of 1.0 (a [1,128] row of ones)
CONSTS_W = C_ONESROW + 128


def build_program(mem_size=MEM_SIZE):
    """Builds the SPMD Bass program (one program, run on all 8 cores)."""
    n_loc = mem_size // N_CORES
    T = n_loc // P
    assert T * P == n_loc
    W = MEM_WIDTH
    TCD = min(8, T)    # DMA chunk (tiles per mem load)
    TC1 = min(4, T)    # phase-1 compute chunk
    TC3 = min(4, T)    # phase-3 chunk
    assert T % TCD == 0 and T % TC1 == 0 and T % TC3 == 0

    nc = bacc.Bacc(
        "TRN2",
        target_bir_lowering=False,
        debug=False,
        num_devices=N_CORES,
    )

    # ---- kernel I/O ----
    mem_in = nc.dram_tensor("mem_in", [n_loc, W], F32, kind="ExternalInput").ap()
    keyb_in = nc.dram_tensor("keyb_in", [P, W], F32, kind="ExternalInput").ap()
    eb_in = nc.dram_tensor("eb_in", [P, W], F32, kind="ExternalInput").ap()
    ab_in = nc.dram_tensor("ab_in", [P, W], F32, kind="ExternalInput").ap()
    pg_in = nc.dram_tensor("pg_in", [P, T], F32, kind="ExternalInput").ap()
    consts_in = nc.dram_tensor("consts_in", [P, CONSTS_W], F32, kind="ExternalInput").ap()
    ident_in = nc.dram_tensor("ident_in", [P, P], F32, kind="ExternalInput").ap()

    newmem_out = nc.dram_tensor("newmem_out", [n_loc, W], F32, kind="ExternalOutput").ap()
    w_out = nc.dram_tensor("w_out", [n_loc], F32, kind="ExternalOutput").ap()
    r_out = nc.dram_tensor("r_out", [W], F32, kind="ExternalOutput").ap()

    mem_in_v = mem_in.rearrange("(p t) c -> p t c", p=P)
    newmem_v = newmem_out.rearrange("(p t) c -> p t c", p=P)
    w_out_v = w_out.rearrange("(p t) -> p t", p=P)

    with tile.TileContext(nc) as tc:
        with (
            tc.tile_pool(name="big", bufs=1) as big,
            tc.tile_pool(name="small", bufs=1) as small,
            tc.tile_pool(name="scratch", bufs=3) as scratch,
            tc.tile_pool(name="sm_scr", bufs=8) as sm_scr,
            tc.tile_pool(name="ps_we", bufs=2, space="PSUM") as ps_we,
            tc.tile_pool(name="ps_wa", bufs=1, space="PSUM") as ps_wa,
            tc.tile_pool(name="ps_r", bufs=1, space="PSUM") as ps_r,
            tc.tile_pool(name="ps_misc", bufs=1, space="PSUM") as ps_misc,
            tc.tile_pool(name="dram", bufs=1, space="DRAM") as dram,
        ):
            # ---- persistent SBUF tensors ----
            mem_sb = big.tile([P, T, W], F32)
            keyb = small.tile([P, W], F32)
            eb = small.tile([P, W], F32)
            ab = small.tile([P, W], F32)
            pg = small.tile([P, T], F32)
            consts = small.tile([P, CONSTS_W], F32)
            ident = small.tile([P, P], F32)
            km = small.tile([P, T], F32)
            nm = small.tile([P, T], F32)
            em = small.tile([P, T], F32)
            wg_ext = small.tile([P, T + 2], F32)
            wlp = small.tile([P, T], F32)
            wfin = small.tile([P, T], F32)
            # bf16 is ample for the rank-1 erase/add outer products: w is
            # ~1e-5 scale, so bf16 rounding perturbs new_mem at ~1e-8.
            wt = small.tile([T, P], BF16)
            eb16 = small.tile([P, W], BF16)
            ab16 = small.tile([P, W], BF16)
            t1ea = small.tile([1, P], F32)
            t1eb = small.tile([1, P], F32)
            ag1 = small.tile([1, 3 * N_CORES], F32)
            ag2 = small.tile([1, N_CORES], F32)

            # small constants / scalars live on partition 0
            ones11 = consts[0:1, C_ONE : C_ONE + 1]
            ones_col = consts[:, C_ONE : C_ONE + 1]
            ones_row = consts[0:1, C_ONESROW : C_ONESROW + P]
            gam_col = consts[:, C_GAMMA : C_GAMMA + 1]
            s0_col = consts[:, C_S0 : C_S0 + 1]
            s1_col = consts[:, C_S1 : C_S1 + 1]
            s2_col = consts[:, C_S2 : C_S2 + 1]

            # ---- load small inputs ----
            nc.sync.dma_start(keyb[:], keyb_in)
            nc.sync.dma_start(eb[:], eb_in)
            nc.sync.dma_start(ab[:], ab_in)
            nc.sync.dma_start(pg[:], pg_in)
            nc.sync.dma_start(consts[:], consts_in)
            nc.sync.dma_start(ident[:], ident_in)

            # ---- DRAM bounce buffers for collectives ----
            agin1 = dram.tile([3], F32)
            agout1 = dram.tile([3 * N_CORES], F32, addr_space="Shared")
            agin2 = dram.tile([1], F32)
            agout2 = dram.tile([N_CORES], F32, addr_space="Shared")

            # =========== phase 1: load mem, row norms + key dot ===========
            for t0 in range(0, T, TCD):
                nc.sync.dma_start(
                    mem_sb[:, t0 : t0 + TCD, :], mem_in_v[:, t0 : t0 + TCD, :]
                )
            keyb_b = keyb.unsqueeze(1).broadcast_to([P, TC1, W])
            for t0 in range(0, T, TC1):
                sl = mem_sb[:, t0 : t0 + TC1, :]
                kk = scratch.tile([P, TC1, W], F32, tag="scr_a", bufs=2)
                nc.vector.tensor_tensor(kk[:], sl, keyb_b, ALU.mult)
                nc.vector.tensor_reduce(km[:, t0 : t0 + TC1], kk[:], X, ALU.add)
                sq = scratch.tile([P, TC1, W], F32, tag="scr_b", bufs=2)
                nc.scalar.activation(sq[:], sl, AF.Square)
                nc.vector.tensor_reduce(nm[:, t0 : t0 + TC1], sq[:], X, ALU.add)

            # =========== phase 2: softmax / gate / shift / sharpen ===========
            # E = exp(dot * beta' * rsqrt(norm2));  rsqrt via exp(-0.5*ln(x))
            ln_n = sm_scr.tile([P, T], F32, tag="sm")
            nc.scalar.activation(ln_n[:], nm[:], AF.Ln)
            rs = sm_scr.tile([P, T], F32, tag="sm")
            nc.scalar.activation(rs[:], ln_n[:], AF.Exp, scale=-0.5)
            th = sm_scr.tile([P, T], F32, tag="sm")
            nc.vector.tensor_mul(th[:], km[:], rs[:])
            nc.scalar.activation(em[:], th[:], AF.Exp)

            # local sum of E -> partition 0 scalar
            ecol = sm_scr.tile([P, 1], F32, tag="sm")
            nc.vector.tensor_reduce(ecol[:], em[:], X, ALU.add)
            sp_ps = ps_misc.tile([1, 1], F32, tag="mm")
            nc.tensor.matmul(sp_ps[:], ecol[:], ones_col, start=True, stop=True)
            spart = sm_scr.tile([1, 1], F32, tag="sm")
            nc.vector.tensor_copy(spart[:], sp_ps[:])

            # AllGather #1: [E_first, E_last, S_partial] per core
            nc.sync.dma_start(agin1[0:1], em[0:1, 0:1])
            nc.sync.dma_start(agin1[1:2], em[P - 1 : P, T - 1 : T])
            nc.sync.dma_start(agin1[2:3], spart[:])
            nc.gpsimd.collective_compute(
                "AllGather",
                ALU.bypass,
                replica_groups=[list(range(N_CORES))],
                ins=[agin1.opt()],
                outs=[agout1.opt()],
            )
            nc.sync.dma_start(ag1[:], agout1[:])

            # S = sum of partial sums (slots 3c+2)
            ag1_v = ag1.rearrange("p (a b) -> p a b", b=3)
            s_tot = sm_scr.tile([1, 1], F32, tag="sm")
            nc.vector.tensor_reduce(s_tot[:], ag1_v[:, :, 2:3], mybir.AxisListType.XY, ALU.add)
            # neighbor halo E values via host-provided one-hot masks
            tl = sm_scr.tile([1, 3 * N_CORES], F32, tag="sm")
            nc.vector.tensor_mul(tl[:], ag1[:], consts[0:1, C_SELL : C_SELL + 3 * N_CORES])
            e_left = sm_scr.tile([1, 1], F32, tag="sm")
            nc.vector.tensor_reduce(e_left[:], tl[:], X, ALU.add)
            tr = sm_scr.tile([1, 3 * N_CORES], F32, tag="sm")
            nc.vector.tensor_mul(tr[:], ag1[:], consts[0:1, C_SELR : C_SELR + 3 * N_CORES])
            e_right = sm_scr.tile([1, 1], F32, tag="sm")
            nc.vector.tensor_reduce(e_right[:], tr[:], X, ALU.add)

            # gs = gate / S, broadcast to all partitions
            s_rec = sm_scr.tile([1, 1], F32, tag="sm")
            nc.vector.reciprocal(s_rec[:], s_tot[:])
            gs = sm_scr.tile([1, 1], F32, tag="sm")
            nc.vector.tensor_mul(gs[:], s_rec[:], consts[0:1, C_GATE : C_GATE + 1])
            gs_ps = ps_misc.tile([P, 1], F32, tag="mm")
            nc.tensor.matmul(gs_ps[:], ones_row, gs[:], start=True, stop=True)
            gs_col = sm_scr.tile([P, 1], F32, tag="sm")
            nc.vector.tensor_copy(gs_col[:], gs_ps[:])

            # wg = gs*E + (1-gate)*clip01(w_prev)   (pg is host-prescaled)
            nc.vector.tensor_scalar(wg_ext[:, 1 : T + 1], em[:], gs_col[:], None, ALU.mult)
            nc.vector.tensor_tensor(wg_ext[:, 1 : T + 1], wg_ext[:, 1 : T + 1], pg[:], ALU.add)

            # ---- circular shift halo columns ----
            # cross-partition: wg_ext[p,0] = wg[p-1,T-1], wg_ext[p,T+1] = wg[p+1,0].
            # Build each halo column as a [1,P] row on partition 0 (free-dim
            # shift + cross-core corner), then matmul it back to a column.
            hgl = sm_scr.tile([1, 1], F32, tag="sm")
            nc.vector.tensor_mul(hgl[:], gs[:], e_left[:])
            nc.vector.tensor_tensor(
                t1ea[0:1, 0:1], hgl[:], consts[0:1, C_HGL : C_HGL + 1], ALU.add
            )
            hgr = sm_scr.tile([1, 1], F32, tag="sm")
            nc.vector.tensor_mul(hgr[:], gs[:], e_right[:])
            nc.vector.tensor_tensor(
                t1eb[0:1, P - 1 : P], hgr[:], consts[0:1, C_HGR : C_HGR + 1], ALU.add
            )

            t1a_ps = ps_misc.tile([1, P], F32, tag="mm")
            nc.tensor.transpose(t1a_ps[:], wg_ext[:, T : T + 1], ident[:])
            nc.vector.tensor_copy(t1ea[0:1, 1:P], t1a_ps[0:1, 0 : P - 1])
            t1b_ps = ps_misc.tile([1, P], F32, tag="mm")
            nc.tensor.transpose(t1b_ps[:], wg_ext[:, 1:2], ident[:])
            nc.vector.tensor_copy(t1eb[0:1, 0 : P - 1], t1b_ps[0:1, 1:P])

            psl = ps_misc.tile([P, 1], F32, tag="mm")
            nc.tensor.matmul(psl[:], t1ea[:], ones11, start=True, stop=True)
            nc.vector.tensor_copy(wg_ext[:, 0:1], psl[:])
            psr = ps_misc.tile([P, 1], F32, tag="mm")
            nc.tensor.matmul(psr[:], t1eb[:], ones11, start=True, stop=True)
            nc.vector.tensor_copy(wg_ext[:, T + 1 : T + 2], psr[:])

            # wl = s0*shift(-1) + s1*id + s2*shift(+1)
            wl = sm_scr.tile([P, T], F32, tag="sm")
            nc.vector.tensor_scalar(wl[:], wg_ext[:, 2 : T + 2], s0_col, None, ALU.mult)
            tmp1 = sm_scr.tile([P, T], F32, tag="sm")
            nc.vector.tensor_scalar(tmp1[:], wg_ext[:, 1 : T + 1], s1_col, None, ALU.mult)
            nc.vector.tensor_tensor(wl[:], wl[:], tmp1[:], ALU.add)
            tmp2 = sm_scr.tile([P, T], F32, tag="sm")
            nc.vector.tensor_scalar(tmp2[:], wg_ext[:, 0:T], s2_col, None, ALU.mult)
            nc.vector.tensor_tensor(wl[:], wl[:], tmp2[:], ALU.add)

            # sharpen: wlp = wl**gamma = exp(gamma*ln(wl))
            lnw = sm_scr.tile([P, T], F32, tag="sm")
            nc.scalar.activation(lnw[:], wl[:], AF.Ln)
            nc.scalar.activation(wlp[:], lnw[:], AF.Exp, scale=gam_col)

            # AllGather #2: global sum of wl**gamma
            s2col = sm_scr.tile([P, 1], F32, tag="sm")
            nc.vector.tensor_reduce(s2col[:], wlp[:], X, ALU.add)
            s2_ps = ps_misc.tile([1, 1], F32, tag="mm")
            nc.tensor.matmul(s2_ps[:], s2col[:], ones_col, start=True, stop=True)
            s2part = sm_scr.tile([1, 1], F32, tag="sm")
            nc.vector.tensor_copy(s2part[:], s2_ps[:])
            nc.sync.dma_start(agin2[0:1], s2part[:])
            nc.gpsimd.collective_compute(
                "AllGather",
                ALU.bypass,
                replica_groups=[list(range(N_CORES))],
                ins=[agin2.opt()],
                outs=[agout2.opt()],
            )
            nc.sync.dma_start(ag2[:], agout2[:])
            s2_tot = sm_scr.tile([1, 1], F32, tag="sm")
            nc.vector.tensor_reduce(s2_tot[:], ag2[:], X, ALU.add)
            s2_rec = sm_scr.tile([1, 1], F32, tag="sm")
            nc.vector.reciprocal(s2_rec[:], s2_tot[:])
            rf_ps = ps_misc.tile([P, 1], F32, tag="mm")
            nc.tensor.matmul(rf_ps[:], ones_row, s2_rec[:], start=True, stop=True)
            rf_col = sm_scr.tile([P, 1], F32, tag="sm")
            nc.vector.tensor_copy(rf_col[:], rf_ps[:])

            # w = min(wlp/S2, 1)   (>=0 already; min guards double-rounding)
            nc.vector.tensor_scalar(wfin[:], wlp[:], rf_col[:], 1.0, ALU.mult, ALU.min)
            nc.sync.dma_start(w_out_v, wfin[:])

            # wt[t,p] = w[p*T+t] (rows of wt are lhsT for the outer products)
            wt_ps = ps_misc.tile([T, P], F32, tag="mm")
            nc.tensor.transpose(wt_ps[:], wfin[:], ident[:])
            nc.vector.tensor_copy(wt[:], wt_ps[:])
            nc.vector.tensor_copy(eb16[:], eb[:])
            nc.vector.tensor_copy(ab16[:], ab[:])

            # =========== phase 3: read vector + erase/add update ===========
            r_ps = ps_r.tile([1, W], F32)
            for t in range(T):
                nc.tensor.matmul(
                    r_ps[:],
                    wfin[:, t : t + 1],
                    mem_sb[:, t, :],
                    start=(t == 0),
                    stop=(t == T - 1),
                )

            # W rows are staged into a small rotating buffer whose two halves
            # sit at partition bases 0 and 64 (legal matmul lhsT bases).
            WT_G = min(8, T)
            half = max(WT_G // 2, 1)
            for g0 in range(0, T, WT_G):
                stage = scratch.tile([65, half * P], BF16, tag="wts", bufs=2)
                stage_v = stage.rearrange("q (g p) -> q g p", p=P)
                nc.sync.dma_start(stage_v[0:1, :, :], wt[g0 : g0 + half, :])
                if WT_G > half:
                    nc.sync.dma_start(
                        stage_v[64:65, :, :], wt[g0 + half : g0 + WT_G, :]
                    )

                def wt_row(t):
                    j = t - g0
                    q = 64 * (j // half)
                    i = j % half
                    return q, stage[q : q + 1, i * P : (i + 1) * P]

                for t0 in range(g0, g0 + WT_G, TC3):
                    we_ps = ps_we.tile([P, TC3, W], F32, tag="we")
                    wa_ps = ps_wa.tile([P, TC3, W], F32, tag="wa")
                    for k in range(TC3):
                        t = t0 + k
                        q, row = wt_row(t)
                        nc.tensor.matmul(
                            we_ps[:, k, :], row, eb16[q : q + 1, :], start=True, stop=True
                        )
                        nc.tensor.matmul(
                            wa_ps[:, k, :], row, ab16[q : q + 1, :], start=True, stop=True
                        )
                    qt = scratch.tile([P, TC3, W], F32, tag="scr3", bufs=3)
                    nc.scalar.activation(
                        qt[:], we_ps[:], AF.Identity, bias=1.0, scale=-1.0
                    )
                    m1 = scratch.tile([P, TC3, W], F32, tag="scr3", bufs=3)
                    nc.vector.tensor_mul(m1[:], mem_sb[:, t0 : t0 + TC3, :], qt[:])
                    # in-place erase/add; WAR deps vs the r-matmul reads are
                    # range-granular so only this chunk's tiles serialize
                    nc.vector.tensor_tensor(
                        mem_sb[:, t0 : t0 + TC3, :], m1[:], wa_ps[:], ALU.add
                    )
                    nc.sync.dma_start(
                        newmem_v[:, t0 : t0 + TC3, :], mem_sb[:, t0 : t0 + TC3, :]
                    )

            r_sb = sm_scr.tile([1, W], F32, tag="smw", bufs=1)
            nc.vector.tensor_copy(r_sb[:], r_ps[:])
            nc.sync.dma_start(r_out, r_sb[:])

    nc.compile()
    return nc, dict(n_loc=n_loc, T=T)


@functools.lru_cache(maxsize=2)
def _get_program(mem_size):
    return build_program(mem_size)


def prepare_in_maps(inputs, mem_size=MEM_SIZE):
    """Host-side preprocessing: clipping, scaling, layout, per-core sharding.

    All arithmetic deliberately in float32 to match the fp32 reference."""
    f32 = np.float32
    mem = np.ascontiguousarray(np.asarray(inputs["mem"], dtype=f32))
    key = np.asarray(inputs["key"], dtype=f32)
    beta = f32(np.asarray(inputs["beta"], dtype=f32))
    gamma = f32(np.asarray(inputs["gamma"], dtype=f32))
    gate = f32(np.asarray(inputs["gate"], dtype=f32))
    shift = np.asarray(inputs["shift"], dtype=f32)
    w_prev = np.asarray(inputs["w_prev"], dtype=f32)
    e = np.asarray(inputs["e"], dtype=f32)
    a = np.asarray(inputs["a"], dtype=f32)

    n_loc = mem_size // N_CORES
    T = n_loc // P

    key_c = np.clip(key, f32(0.0), f32(1.0)).astype(f32)
    norm_key = f32(np.sqrt(np.sum(key_c * key_c, dtype=f32)))
    key_s = (key_c * f32(beta / norm_key)).astype(f32)
    e_c = np.clip(e, f32(0.0), f32(1.0)).astype(f32)
    a_c = np.clip(a, f32(0.0), f32(1.0)).astype(f32)
    p_c = np.clip(w_prev, f32(0.0), f32(1.0)).astype(f32)
    s_c = np.clip(shift, f32(0.0), f32(1.0)).astype(f32)
    one_m_gate = f32(f32(1.0) - gate)
    pg_full = (one_m_gate * p_c).astype(f32)

    keyb = np.tile(key_s[None, :], (P, 1))
    eb = np.tile(e_c[None, :], (P, 1))
    ab = np.tile(a_c[None, :], (P, 1))

    in_maps = []
    for c in range(N_CORES):
        r0 = c * n_loc
        consts = np.zeros((CONSTS_W,), dtype=f32)
        consts[C_ONE] = 1.0
        consts[C_GAMMA] = gamma
        consts[C_S0] = s_c[0]
        consts[C_S1] = s_c[1]
        consts[C_S2] = s_c[2]
        consts[C_HGL] = pg_full[(r0 - 1) % mem_size]
        consts[C_HGR] = pg_full[(r0 + n_loc) % mem_size]
        consts[C_GATE] = gate
        consts[C_SELL + (((c - 1) % N_CORES) * 3 + 1)] = 1.0
        consts[C_SELR + (((c + 1) % N_CORES) * 3 + 0)] = 1.0
        consts[C_ONESROW : C_ONESROW + P] = 1.0
        in_maps.append(
            {
                "mem_in": mem[r0 : r0 + n_loc],
                "keyb_in": keyb,
                "eb_in": eb,
                "ab_in": ab,
                "pg_in": pg_full[r0 : r0 + n_loc].reshape(P, T),
                "consts_in": np.tile(consts[None, :], (P, 1)),
                "ident_in": np.eye(P, dtype=f32),
            }
        )
    return in_maps


def assemble_outputs(results, mem_size=MEM_SIZE):
    new_mem = np.concatenate([res["newmem_out"] for res in results], axis=0)
    w = np.concatenate([res["w_out"] for res in results], axis=0)
    r = np.sum(np.stack([res["r_out"] for res in results]), axis=0, dtype=np.float32)
    return r.astype(np.float32), new_mem, w


def run(inputs, mem_size=MEM_SIZE, trace=False, trace_kwargs=None):
    nc, _meta = _get_program(mem_size)
    in_maps = prepare_in_maps(inputs, mem_size)
    res = run_bass_kernel_spmd(
        nc,
        in_maps,
        list(range(N_CORES)),
        trace=trace,
        **(trace_kwargs or {}),
    )
    outs = assemble_outputs(res.results, mem_size)
    return outs, res


def kernel(**inputs):
    outs, _res = run(inputs)
    return outs


# revision 30
# speedup vs baseline: 1.1131x; 1.1131x over previous
"""NTM external-memory step (cosine addressing -> shift -> sharpen -> read/erase/add)
as a Bass/Tile SPMD kernel for 8 Trainium2 NeuronCores.

Sharding: mem (and all N-length address vectors) are sharded along the
mem_size axis across the 8 cores.  Each core keeps its 16 MiB mem shard
resident in SBUF between the addressing pass and the erase/add pass, so HBM
traffic is one read + one write of mem.  The softmax / sharpen
normalizations and the circular-shift halo are handled with two tiny
AllGather collectives (3 floats and 1 float per core).

Per-core layout (n_loc = mem_size/8 rows, P=128 partitions, T = n_loc/128):
  local row l = p*T + t  lives at SBUF partition p, free block t.
  Address vectors are [128, T] tensors; the +-1 circular shift is a free-dim
  offset except at per-partition block boundaries, which are patched via two
  PE transpose round-trips, and at core boundaries, which come from the
  AllGather halo.
"""

import sys
import functools

for _p in ("/opt/trn_rl_repo",):
    if _p not in sys.path:
        sys.path.insert(0, _p)

import numpy as np

from concourse import bass, bacc, mybir, tile
from concourse.bass_utils import run_bass_kernel_spmd

F32 = mybir.dt.float32
BF16 = mybir.dt.bfloat16
AF = mybir.ActivationFunctionType
ALU = mybir.AluOpType
X = mybir.AxisListType.X

N_CORES = 8
MEM_SIZE = 131072
MEM_WIDTH = 256
P = 128

# consts column map (consts input is [128, CONSTS_W], replicated across partitions)
C_ONE = 0
C_GAMMA = 1
C_S0 = 2
C_S1 = 3
C_S2 = 4
C_HGL = 5   # (1-gate)*clip01(w_prev[left halo row])
C_HGR = 6   # (1-gate)*clip01(w_prev[right halo row])
C_GATE = 7
C_SELL = 8                      # 3*N_CORES one-hot mask for left-neighbor E_last
C_SELR = 8 + 3 * N_CORES        # 3*N_CORES one-hot mask for right-neighbor E_first
C_ONESROW = 8 + 6 * N_CORES     # 128 columns of 1.0 (a [1,128] row of ones)
CONSTS_W = C_ONESROW + 128


def build_program(mem_size=MEM_SIZE):
    """Builds the SPMD Bass program (one program, run on all 8 cores)."""
    n_loc = mem_size // N_CORES
    T = n_loc // P
    assert T * P == n_loc
    W = MEM_WIDTH
    TCD = min(16, T)   # DMA chunk (tiles per mem load/store)
    TC1 = min(4, T)    # phase-1 compute chunk
    TC3 = min(4, T)    # phase-3 chunk
    assert T % TCD == 0 and T % TC1 == 0 and T % TC3 == 0

    nc = bacc.Bacc(
        "TRN2",
        target_bir_lowering=False,
        debug=False,
        num_devices=N_CORES,
    )

    # ---- kernel I/O ----
    mem_in = nc.dram_tensor("mem_in", [n_loc, W], F32, kind="ExternalInput").ap()
    keyb_in = nc.dram_tensor("keyb_in", [P, W], F32, kind="ExternalInput").ap()
    eb_in = nc.dram_tensor("eb_in", [P, W], F32, kind="ExternalInput").ap()
    ab_in = nc.dram_tensor("ab_in", [P, W], F32, kind="ExternalInput").ap()
    pg_in = nc.dram_tensor("pg_in", [P, T], F32, kind="ExternalInput").ap()
    consts_in = nc.dram_tensor("consts_in", [P, CONSTS_W], F32, kind="ExternalInput").ap()
    ident_in = nc.dram_tensor("ident_in", [P, P], F32, kind="ExternalInput").ap()

    newmem_out = nc.dram_tensor("newmem_out", [n_loc, W], F32, kind="ExternalOutput").ap()
    w_out = nc.dram_tensor("w_out", [n_loc], F32, kind="ExternalOutput").ap()
    r_out = nc.dram_tensor("r_out", [W], F32, kind="ExternalOutput").ap()

    mem_in_v = mem_in.rearrange("(p t) c -> p t c", p=P)
    newmem_v = newmem_out.rearrange("(p t) c -> p t c", p=P)
    w_out_v = w_out.rearrange("(p t) -> p t", p=P)

    with tile.TileContext(nc) as tc:
        with (
            tc.tile_pool(name="big", bufs=1) as big,
            tc.tile_pool(name="small", bufs=1) as small,
            tc.tile_pool(name="scratch", bufs=3) as scratch,
            tc.tile_pool(name="sm_scr", bufs=8) as sm_scr,
            tc.tile_pool(name="ps_we", bufs=2, space="PSUM") as ps_we,
            tc.tile_pool(name="ps_wa", bufs=1, space="PSUM") as ps_wa,
            tc.tile_pool(name="ps_r", bufs=1, space="PSUM") as ps_r,
            tc.tile_pool(name="ps_misc", bufs=1, space="PSUM") as ps_misc,
            tc.tile_pool(name="dram", bufs=1, space="DRAM") as dram,
        ):
            # ---- persistent SBUF tensors ----
            mem_sb = big.tile([P, T, W], F32)
            keyb = small.tile([P, W], F32)
            eb = small.tile([P, W], F32)
            ab = small.tile([P, W], F32)
            pg = small.tile([P, T], F32)
            consts = small.tile([P, CONSTS_W], F32)
            ident = small.tile([P, P], F32)
            km = small.tile([P, T], F32)
            nm = small.tile([P, T], F32)
            em = small.tile([P, T], F32)
            wg_ext = small.tile([P, T + 2], F32)
            wlp = small.tile([P, T], F32)
            wfin = small.tile([P, T], F32)
            # bf16 is ample for the rank-1 erase/add outer products: w is
            # ~1e-5 scale, so bf16 rounding perturbs new_mem at ~1e-8.
            wt = small.tile([T, P], BF16)
            eb16 = small.tile([P, W], BF16)
            ab16 = small.tile([P, W], BF16)
            t1ea = small.tile([1, P], F32)
            t1eb = small.tile([1, P], F32)
            ag1 = small.tile([1, 3 * N_CORES], F32)
            ag2 = small.tile([1, N_CORES], F32)

            # small constants / scalars live on partition 0
            ones11 = consts[0:1, C_ONE : C_ONE + 1]
            ones_col = consts[:, C_ONE : C_ONE + 1]
            ones_row = consts[0:1, C_ONESROW : C_ONESROW + P]
            gam_col = consts[:, C_GAMMA : C_GAMMA + 1]
            s0_col = consts[:, C_S0 : C_S0 + 1]
            s1_col = consts[:, C_S1 : C_S1 + 1]
            s2_col = consts[:, C_S2 : C_S2 + 1]

            # ---- load small inputs ----
            nc.sync.dma_start(keyb[:], keyb_in)
            nc.sync.dma_start(eb[:], eb_in)
            nc.sync.dma_start(ab[:], ab_in)
            nc.sync.dma_start(pg[:], pg_in)
            nc.sync.dma_start(consts[:], consts_in)
            nc.sync.dma_start(ident[:], ident_in)
            nc.vector.tensor_copy(eb16[:], eb[:])
            nc.vector.tensor_copy(ab16[:], ab[:])

            # ---- DRAM bounce buffers for collectives ----
            agin1 = dram.tile([3], F32)
            agout1 = dram.tile([3 * N_CORES], F32, addr_space="Shared")
            agin2 = dram.tile([1], F32)
            agout2 = dram.tile([N_CORES], F32, addr_space="Shared")

            # =========== phase 1: load mem, row norms + key dot ===========
            for t0 in range(0, T, TCD):
                nc.sync.dma_start(
                    mem_sb[:, t0 : t0 + TCD, :], mem_in_v[:, t0 : t0 + TCD, :]
                )
            keyb_b = keyb.unsqueeze(1).broadcast_to([P, TC1, W])
            for t0 in range(0, T, TC1):
                sl = mem_sb[:, t0 : t0 + TC1, :]
                kk = scratch.tile([P, TC1, W], F32, tag="scr_a", bufs=2)
                nc.gpsimd.tensor_tensor(kk[:], sl, keyb_b, ALU.mult)
                nc.vector.tensor_reduce(km[:, t0 : t0 + TC1], kk[:], X, ALU.add)
                sq = scratch.tile([P, TC1, W], F32, tag="scr_b", bufs=2)
                nc.scalar.activation(sq[:], sl, AF.Square)
                nc.vector.tensor_reduce(nm[:, t0 : t0 + TC1], sq[:], X, ALU.add)

            # =========== phase 2: softmax / gate / shift / sharpen ===========
            # E = exp(dot * beta' * rsqrt(norm2));  rsqrt via exp(-0.5*ln(x))
            ln_n = sm_scr.tile([P, T], F32, tag="sm")
            nc.scalar.activation(ln_n[:], nm[:], AF.Ln)
            rs = sm_scr.tile([P, T], F32, tag="sm")
            nc.scalar.activation(rs[:], ln_n[:], AF.Exp, scale=-0.5)
            th = sm_scr.tile([P, T], F32, tag="sm")
            nc.vector.tensor_mul(th[:], km[:], rs[:])
            nc.scalar.activation(em[:], th[:], AF.Exp)

            # local sum of E -> partition 0 scalar
            ecol = sm_scr.tile([P, 1], F32, tag="sm")
            nc.vector.tensor_reduce(ecol[:], em[:], X, ALU.add)
            sp_ps = ps_misc.tile([1, 1], F32, tag="mm")
            nc.tensor.matmul(sp_ps[:], ecol[:], ones_col, start=True, stop=True)
            spart = sm_scr.tile([1, 1], F32, tag="sm")
            nc.vector.tensor_copy(spart[:], sp_ps[:])

            # AllGather #1: [E_first, E_last, S_partial] per core
            nc.sync.dma_start(agin1[0:1], em[0:1, 0:1])
            nc.sync.dma_start(agin1[1:2], em[P - 1 : P, T - 1 : T])
            nc.sync.dma_start(agin1[2:3], spart[:])
            nc.gpsimd.collective_compute(
                "AllGather",
                ALU.bypass,
                replica_groups=[list(range(N_CORES))],
                ins=[agin1.opt()],
                outs=[agout1.opt()],
            )
            nc.sync.dma_start(ag1[:], agout1[:])

            # S = sum of partial sums (slots 3c+2)
            ag1_v = ag1.rearrange("p (a b) -> p a b", b=3)
            s_tot = sm_scr.tile([1, 1], F32, tag="sm")
            nc.vector.tensor_reduce(s_tot[:], ag1_v[:, :, 2:3], mybir.AxisListType.XY, ALU.add)
            # neighbor halo E values via host-provided one-hot masks
            tl = sm_scr.tile([1, 3 * N_CORES], F32, tag="sm")
            nc.vector.tensor_mul(tl[:], ag1[:], consts[0:1, C_SELL : C_SELL + 3 * N_CORES])
            e_left = sm_scr.tile([1, 1], F32, tag="sm")
            nc.vector.tensor_reduce(e_left[:], tl[:], X, ALU.add)
            tr = sm_scr.tile([1, 3 * N_CORES], F32, tag="sm")
            nc.vector.tensor_mul(tr[:], ag1[:], consts[0:1, C_SELR : C_SELR + 3 * N_CORES])
            e_right = sm_scr.tile([1, 1], F32, tag="sm")
            nc.vector.tensor_reduce(e_right[:], tr[:], X, ALU.add)

            # gs = gate / S, broadcast to all partitions
            s_rec = sm_scr.tile([1, 1], F32, tag="sm")
            nc.vector.reciprocal(s_rec[:], s_tot[:])
            gs = sm_scr.tile([1, 1], F32, tag="sm")
            nc.vector.tensor_mul(gs[:], s_rec[:], consts[0:1, C_GATE : C_GATE + 1])
            gs_ps = ps_misc.tile([P, 1], F32, tag="mm")
            nc.tensor.matmul(gs_ps[:], ones_row, gs[:], start=True, stop=True)
            gs_col = sm_scr.tile([P, 1], F32, tag="sm")
            nc.vector.tensor_copy(gs_col[:], gs_ps[:])

            # wg = gs*E + (1-gate)*clip01(w_prev)   (pg is host-prescaled)
            nc.vector.tensor_scalar(wg_ext[:, 1 : T + 1], em[:], gs_col[:], None, ALU.mult)
            nc.vector.tensor_tensor(wg_ext[:, 1 : T + 1], wg_ext[:, 1 : T + 1], pg[:], ALU.add)

            # ---- circular shift halo columns ----
            # cross-partition: wg_ext[p,0] = wg[p-1,T-1], wg_ext[p,T+1] = wg[p+1,0].
            # Build each halo column as a [1,P] row on partition 0 (free-dim
            # shift + cross-core corner), then matmul it back to a column.
            hgl = sm_scr.tile([1, 1], F32, tag="sm")
            nc.vector.tensor_mul(hgl[:], gs[:], e_left[:])
            nc.vector.tensor_tensor(
                t1ea[0:1, 0:1], hgl[:], consts[0:1, C_HGL : C_HGL + 1], ALU.add
            )
            hgr = sm_scr.tile([1, 1], F32, tag="sm")
            nc.vector.tensor_mul(hgr[:], gs[:], e_right[:])
            nc.vector.tensor_tensor(
                t1eb[0:1, P - 1 : P], hgr[:], consts[0:1, C_HGR : C_HGR + 1], ALU.add
            )

            t1a_ps = ps_misc.tile([1, P], F32, tag="mm")
            nc.tensor.transpose(t1a_ps[:], wg_ext[:, T : T + 1], ident[:])
            nc.vector.tensor_copy(t1ea[0:1, 1:P], t1a_ps[0:1, 0 : P - 1])
            t1b_ps = ps_misc.tile([1, P], F32, tag="mm")
            nc.tensor.transpose(t1b_ps[:], wg_ext[:, 1:2], ident[:])
            nc.vector.tensor_copy(t1eb[0:1, 0 : P - 1], t1b_ps[0:1, 1:P])

            psl = ps_misc.tile([P, 1], F32, tag="mm")
            nc.tensor.matmul(psl[:], t1ea[:], ones11, start=True, stop=True)
            nc.vector.tensor_copy(wg_ext[:, 0:1], psl[:])
            psr = ps_misc.tile([P, 1], F32, tag="mm")
            nc.tensor.matmul(psr[:], t1eb[:], ones11, start=True, stop=True)
            nc.vector.tensor_copy(wg_ext[:, T + 1 : T + 2], psr[:])

            # wl = s0*shift(-1) + s1*id + s2*shift(+1)
            wl = sm_scr.tile([P, T], F32, tag="sm")
            nc.vector.tensor_scalar(wl[:], wg_ext[:, 2 : T + 2], s0_col, None, ALU.mult)
            tmp1 = sm_scr.tile([P, T], F32, tag="sm")
            nc.vector.tensor_scalar(tmp1[:], wg_ext[:, 1 : T + 1], s1_col, None, ALU.mult)
            nc.vector.tensor_tensor(wl[:], wl[:], tmp1[:], ALU.add)
            tmp2 = sm_scr.tile([P, T], F32, tag="sm")
            nc.vector.tensor_scalar(tmp2[:], wg_ext[:, 0:T], s2_col, None, ALU.mult)
            nc.vector.tensor_tensor(wl[:], wl[:], tmp2[:], ALU.add)

            # sharpen: wlp = wl**gamma = exp(gamma*ln(wl))
            lnw = sm_scr.tile([P, T], F32, tag="sm")
            nc.scalar.activation(lnw[:], wl[:], AF.Ln)
            nc.scalar.activation(wlp[:], lnw[:], AF.Exp, scale=gam_col)

            # AllGather #2: global sum of wl**gamma
            s2col = sm_scr.tile([P, 1], F32, tag="sm")
            nc.vector.tensor_reduce(s2col[:], wlp[:], X, ALU.add)
            s2_ps = ps_misc.tile([1, 1], F32, tag="mm")
            nc.tensor.matmul(s2_ps[:], s2col[:], ones_col, start=True, stop=True)
            s2part = sm_scr.tile([1, 1], F32, tag="sm")
            nc.vector.tensor_copy(s2part[:], s2_ps[:])
            nc.sync.dma_start(agin2[0:1], s2part[:])
            nc.gpsimd.collective_compute(
                "AllGather",
                ALU.bypass,
                replica_groups=[list(range(N_CORES))],
                ins=[agin2.opt()],
                outs=[agout2.opt()],
            )
            nc.sync.dma_start(ag2[:], agout2[:])

            # ---- AG2-independent work (overlaps the collective) ----
            # PE computes r' = mem.T @ wlp and wE' = wlp (x) e on the
            # UNNORMALIZED sharpened weights; 1/S2 is folded in afterwards
            # (ACT scale for the erase term, pre-scaled `a` for the add term,
            # and a final [1,W] scale for r).
            wt_ps = ps_misc.tile([T, P], F32, tag="mm")
            nc.tensor.transpose(wt_ps[:], wlp[:], ident[:])
            nc.vector.tensor_copy(wt[:], wt_ps[:])

            r_ps = ps_r.tile([1, W], F32)
            for t in range(T):
                nc.tensor.matmul(
                    r_ps[:],
                    wlp[:, t : t + 1],
                    mem_sb[:, t, :],
                    start=(t == 0),
                    stop=(t == T - 1),
                )

            # ---- AG2 landing: normalization factors ----
            s2_tot = sm_scr.tile([1, 1], F32, tag="sm")
            nc.vector.tensor_reduce(s2_tot[:], ag2[:], X, ALU.add)
            s2_rec = sm_scr.tile([1, 1], F32, tag="sm")
            nc.vector.reciprocal(s2_rec[:], s2_tot[:])
            rf_ps = ps_misc.tile([P, 1], F32, tag="mm")
            nc.tensor.matmul(rf_ps[:], ones_row, s2_rec[:], start=True, stop=True)
            rf_col = sm_scr.tile([P, 1], F32, tag="sm")
            nc.vector.tensor_copy(rf_col[:], rf_ps[:])
            nrf_col = sm_scr.tile([P, 1], F32, tag="sm")
            nc.vector.tensor_scalar(nrf_col[:], rf_col[:], -1.0, None, ALU.mult)
            ab16c = sm_scr.tile([P, W], BF16, tag="smw2", bufs=1)
            nc.vector.tensor_scalar(ab16c[:], ab16[:], rf_col[:], None, ALU.mult)

            # w = min(wlp/S2, 1)   (>=0 already; min guards double-rounding)
            nc.vector.tensor_scalar(wfin[:], wlp[:], rf_col[:], 1.0, ALU.mult, ALU.min)
            nc.sync.dma_start(w_out_v, wfin[:])

            # =========== phase 3: erase/add update ===========
            # W rows are staged into a small rotating buffer whose two halves
            # sit at partition bases 0 and 64 (legal matmul lhsT bases).
            WT_G = min(16, T)
            half = max(WT_G // 2, 1)
            for g0 in range(0, T, WT_G):
                stage = scratch.tile([65, half * P], BF16, tag="wts", bufs=2)
                stage_v = stage.rearrange("q (g p) -> q g p", p=P)
                nc.sync.dma_start(stage_v[0:1, :, :], wt[g0 : g0 + half, :])
                if WT_G > half:
                    nc.sync.dma_start(
                        stage_v[64:65, :, :], wt[g0 + half : g0 + WT_G, :]
                    )

                def wt_row(t):
                    j = t - g0
                    q = 64 * (j // half)
                    i = j % half
                    return q, stage[q : q + 1, i * P : (i + 1) * P]

                for t0 in range(g0, g0 + WT_G, TC3):
                    we_ps = ps_we.tile([P, TC3, W], F32, tag="we")
                    wa_ps = ps_wa.tile([P, TC3, W], F32, tag="wa")
                    for k in range(TC3):
                        t = t0 + k
                        q, row = wt_row(t)
                        nc.tensor.matmul(
                            we_ps[:, k, :], row, eb16[q : q + 1, :], start=True, stop=True
                        )
                        nc.tensor.matmul(
                            wa_ps[:, k, :], row, ab16c[q : q + 1, :], start=True, stop=True
                        )
                    # q = 1 - wE'/S2 via the free affine on ACT
                    qt = scratch.tile([P, TC3, W], F32, tag="scr3", bufs=3)
                    nc.scalar.activation(
                        qt[:], we_ps[:], AF.Identity, bias=1.0, scale=nrf_col[:]
                    )
                    m1 = scratch.tile([P, TC3, W], F32, tag="scr3", bufs=3)
                    nc.vector.tensor_mul(m1[:], mem_sb[:, t0 : t0 + TC3, :], qt[:])
                    # in-place erase/add; WAR deps vs the r-matmul reads are
                    # range-granular so only this chunk's tiles serialize
                    nc.vector.tensor_tensor(
                        mem_sb[:, t0 : t0 + TC3, :], m1[:], wa_ps[:], ALU.add
                    )
                # batched store per staging group
                nc.sync.dma_start(
                    newmem_v[:, g0 : g0 + WT_G, :], mem_sb[:, g0 : g0 + WT_G, :]
                )

            r_sb = sm_scr.tile([1, W], F32, tag="smw", bufs=1)
            nc.vector.tensor_copy(r_sb[:], r_ps[:])
            nc.vector.tensor_scalar(r_sb[:], r_sb[:], s2_rec[0:1, :], None, ALU.mult)
            nc.sync.dma_start(r_out, r_sb[:])

    nc.compile()
    return nc, dict(n_loc=n_loc, T=T)


@functools.lru_cache(maxsize=2)
def _get_program(mem_size):
    return build_program(mem_size)


def prepare_in_maps(inputs, mem_size=MEM_SIZE):
    """Host-side preprocessing: clipping, scaling, layout, per-core sharding.

    All arithmetic deliberately in float32 to match the fp32 reference."""
    f32 = np.float32
    mem = np.ascontiguousarray(np.asarray(inputs["mem"], dtype=f32))
    key = np.asarray(inputs["key"], dtype=f32)
    beta = f32(np.asarray(inputs["beta"], dtype=f32))
    gamma = f32(np.asarray(inputs["gamma"], dtype=f32))
    gate = f32(np.asarray(inputs["gate"], dtype=f32))
    shift = np.asarray(inputs["shift"], dtype=f32)
    w_prev = np.asarray(inputs["w_prev"], dtype=f32)
    e = np.asarray(inputs["e"], dtype=f32)
    a = np.asarray(inputs["a"], dtype=f32)

    n_loc = mem_size // N_CORES
    T = n_loc // P

    key_c = np.clip(key, f32(0.0), f32(1.0)).astype(f32)
    norm_key = f32(np.sqrt(np.sum(key_c * key_c, dtype=f32)))
    key_s = (key_c * f32(beta / norm_key)).astype(f32)
    e_c = np.clip(e, f32(0.0), f32(1.0)).astype(f32)
    a_c = np.clip(a, f32(0.0), f32(1.0)).astype(f32)
    p_c = np.clip(w_prev, f32(0.0), f32(1.0)).astype(f32)
    s_c = np.clip(shift, f32(0.0), f32(1.0)).astype(f32)
    one_m_gate = f32(f32(1.0) - gate)
    pg_full = (one_m_gate * p_c).astype(f32)

    keyb = np.tile(key_s[None, :], (P, 1))
    eb = np.tile(e_c[None, :], (P, 1))
    ab = np.tile(a_c[None, :], (P, 1))

    in_maps = []
    for c in range(N_CORES):
        r0 = c * n_loc
        consts = np.zeros((CONSTS_W,), dtype=f32)
        consts[C_ONE] = 1.0
        consts[C_GAMMA] = gamma
        consts[C_S0] = s_c[0]
        consts[C_S1] = s_c[1]
        consts[C_S2] = s_c[2]
        consts[C_HGL] = pg_full[(r0 - 1) % mem_size]
        consts[C_HGR] = pg_full[(r0 + n_loc) % mem_size]
        consts[C_GATE] = gate
        consts[C_SELL + (((c - 1) % N_CORES) * 3 + 1)] = 1.0
        consts[C_SELR + (((c + 1) % N_CORES) * 3 + 0)] = 1.0
        consts[C_ONESROW : C_ONESROW + P] = 1.0
        in_maps.append(
            {
                "mem_in": mem[r0 : r0 + n_loc],
                "keyb_in": keyb,
                "eb_in": eb,
                "ab_in": ab,
                "pg_in": pg_full[r0 : r0 + n_loc].reshape(P, T),
                "consts_in": np.tile(consts[None, :], (P, 1)),
                "ident_in": np.eye(P, dtype=f32),
            }
        )
    return in_maps


def assemble_outputs(results, mem_size=MEM_SIZE):
    new_mem = np.concatenate([res["newmem_out"] for res in results], axis=0)
    w = np.concatenate([res["w_out"] for res in results], axis=0)
    r = np.sum(np.stack([res["r_out"] for res in results]), axis=0, dtype=np.float32)
    return r.astype(np.float32), new_mem, w


def run(inputs, mem_size=MEM_SIZE, trace=False, trace_kwargs=None):
    nc, _meta = _get_program(mem_size)
    in_maps = prepare_in_maps(inputs, mem_size)
    res = run_bass_kernel_spmd(
        nc,
        in_maps,
        list(range(N_CORES)),
        trace=trace,
        **(trace_kwargs or {}),
    )
    outs = assemble_outputs(res.results, mem_size)
    return outs, res


def kernel(**inputs):
    outs, _res = run(inputs)
    return outs
